# revision 2
# baseline (speedup 1.0000x reference)
"""Trainium2 Bass kernel for histogram_binning — single fused DVE pass.

Reference (per element):
    d = x[k,i] - phis[i,j]
    out[k, i*L+j] = 0.5*cos(d)+0.5  if  -interval[i] < d <= interval[i]  else 0

Design (8 cores, data-parallel over batch; per-core out shard [128, 65536]):
  - fp16 output, host-upcast to f32 (rel-err gate 2e-2; fp16 + poly error
    ~8e-4).  Halves HBM write traffic vs f32.
  - Partition p owns feature pair {2p, 2p+1} (t=0/1), so each DMA line for
    (p, k) covers t*256+j = 512 fp16 = 1KB contiguous DRAM (512B lines are
    descriptor-bound; 1KB halves descriptor count -> DMA ~48us/core).
  - For |d| < interval[i] < 1:  0.5*cos(d)+0.5 = (1 - d^2/8)^2 + e,
    |e| <= d^4/192 < 4.6e-3, so ONE 7-stage custom DVE op per (chunk, t)
    computes window + value + mask + fp16 cast at 1 elem/cycle:
        d = Src1 - Src0   (x - phi; Src0 = phi bcast over k,
        u = d*d            Src1 = x bcast over j; stride-0 APs)
        c = u <= C0        (C0 = iv^2 [P,1]; boundary-rounding flips ~1e-7)
        t = One - C1*u     (C1 = 1/8)
        out = (c*t)*t -> fp16
    No ACT, no post pass, no GPSIMD (GPSIMD contends for the DVE SBUF
    port and poisons throughput).  DVE ~69us/core is the structural floor:
    combining the (i,j)- and (k,i)-indexed streams is inherently a 2-port
    DVE op, which caps at 1 elem/cycle.
"""

import os

import numpy as np

import concourse.bacc as bacc
import concourse.mybir as mybir
from concourse import dve_ops
from concourse.bass_utils import run_bass_kernel_spmd
from concourse.dve_spec import C0, C1, One, Spec, Src0, Src1, _has_src1, lower
from concourse.dve_uop import DveOpSpec
from concourse.tile import TileContext

B, M, L = 1024, 256, 256
N_CORES = 8
B_SHARD = B // N_CORES  # 128
F32 = mybir.dt.float32
F16 = mybir.dt.float16
P = 128
T = 2

_OPS_CACHE = {}


def _register_op(name, spec):
    """Register a custom DVE op under `name`, computing its uops sha."""
    if name in _OPS_CACHE:
        return _OPS_CACHE[name]
    for existing in dve_ops.OPS:
        if existing.name == name:
            _OPS_CACHE[name] = existing
            return existing
    if name not in dve_ops._SUB_OPCODE_FOR_NAME:
        row = max(dve_ops._SUB_OPCODE_FOR_NAME.values()) + 1
        assert row < 0x20, "no free custom-DVE opcode rows"
        dve_ops._SUB_OPCODE_FOR_NAME[name] = row
    shas = {}
    for ver in ("v3", "v4"):
        uops = lower(spec, ver=ver)
        shas[ver] = DveOpSpec(
            name=name,
            opcode=dve_ops.get_dve_sub_opcode(name),
            uops=uops,
            rd1_en=_has_src1(spec),
        ).sha(ver)
    op = dve_ops.DveOp(name, spec, subdim=False, uops_sha=shas)
    dve_ops.OPS.append(op)
    dve_ops.CUSTOM_DVE_SPECS[name] = spec
    _OPS_CACHE[name] = op
    return op


def _get_pwinpoly_op():
    """Window via squared compare (C0 = iv^2): d^2 <= iv^2 differs from the
    exact (-iv, iv] window only at fp32-rounding boundary flips (~1e-7 of
    elements) and at d == -iv exactly (measure zero)."""
    d = Src1 - Src0
    u = d * d
    cond = u <= C0
    t = One - C1 * u
    body = (cond * t) * t

    def _ref(in0, in1, s0, s1, imm2):
        f = np.float32
        dd = (in1 - in0).astype(np.float32)
        u = (dd * dd).astype(np.float32)
        m = u <= s0
        t = (f(1.0) - (f(s1) * u).astype(np.float32)).astype(np.float32)
        return ((m.astype(np.float32) * t) * t).astype(np.float32)

    return _register_op("PWINPOLY_ANT", Spec(body=body, reference=_ref))


def build_nc(
    K=8,
    num_devices=N_CORES,
    bufs=None,
    reps=1,
    timing=False,
):
    assert B_SHARD % K == 0
    n_chunks = B_SHARD // K

    nc = bacc.Bacc(
        "TRN2",
        target_bir_lowering=False,
        debug=False,
        enable_asserts=True,
        num_devices=num_devices,
    )
    xt_d = nc.dram_tensor("xt", [M, B_SHARD], F32, kind="ExternalInput")
    ph_d = nc.dram_tensor("phis", [M, L], F32, kind="ExternalInput")
    iv_d = nc.dram_tensor("interval", [M], F32, kind="ExternalInput")
    out_kind = "Internal" if timing else "ExternalOutput"
    y_d = nc.dram_tensor("out", [B_SHARD, M * L], F16, kind=out_kind)
    sink_d = (
        nc.dram_tensor("sink", [P, 1], F32, kind="ExternalOutput")
        if timing
        else None
    )
    # out[b, 512p + 256t + j] viewed as [p, b, (t j)]
    yr = y_d.ap().rearrange("b (p t j) -> p b (t j)", p=P, t=T, j=L)
    phr = ph_d.ap().rearrange("(p t) j -> p (t j)", t=T)
    ivr = iv_d.ap().rearrange("(p t) -> p t", t=T)
    xtr = xt_d.ap().rearrange("(p t) (ci k) -> p ci t k", t=T, k=K)

    pwin = _get_pwinpoly_op()

    if bufs is None:
        bufs = 4 if K <= 16 else 3
    with TileContext(nc) as tc:
        with (
            tc.tile_pool(name="const", bufs=1) as cpool,
            tc.tile_pool(name="owork", bufs=bufs) as opool,
        ):
            ph_t = cpool.tile([P, T * L], F32, tag="ph")
            nc.sync.dma_start(out=ph_t[:], in_=phr)
            iv_t = cpool.tile([P, T], F32, tag="iv")
            nc.sync.dma_start(out=iv_t[:], in_=ivr)
            iv2_t = cpool.tile([P, T], F32, tag="iv2")
            nc.vector.tensor_tensor(
                out=iv2_t[:], in0=iv_t[:], in1=iv_t[:],
                op=mybir.AluOpType.mult,
            )
            xk_t = cpool.tile([P, T * B_SHARD], F32, tag="xk")
            xk_v = xk_t[:].rearrange("p (ci t k) -> p ci t k", t=T, k=K)
            for t in range(T):
                nc.sync.dma_start(out=xk_v[:, :, t, :], in_=xtr[:, :, t, :])

            def emit_chunk(ci):
                o = opool.tile([P, K * T * L], F16, tag="o")
                o4 = o[:].rearrange("p (k t j) -> p k t j", t=T, j=L)
                for t in range(T):
                    in0 = (
                        ph_t[:, t * L : (t + 1) * L]
                        .rearrange("p (o j) -> p o j", o=1)
                        .broadcast_to((P, K, L))
                    )
                    in1 = (
                        xk_t[:, ci * T * K + t * K : ci * T * K + (t + 1) * K]
                        .rearrange("p (k o) -> p k o", o=1)
                        .broadcast_to((P, K, L))
                    )
                    nc.vector._custom_dve(
                        pwin,
                        out=o4[:, :, t, :],
                        in0=in0,
                        in1=in1,
                        s0=iv2_t[:, t : t + 1],
                        s1=0.125,
                    )
                nc.sync.dma_start(
                    out=yr[:, ci * K : (ci + 1) * K, :], in_=o[:]
                )

            import contextlib

            loop_ctx = (
                tc.For_i(0, reps, 1, hint_engines=tuple(mybir.ALL_ENGINES))
                if reps > 1
                else contextlib.nullcontext()
            )
            with loop_ctx:
                for ci in range(n_chunks):
                    emit_chunk(ci)
            if timing:
                nc.sync.dma_start(out=sink_d.ap(), in_=iv_t[:, 0:1])
    nc.compile()
    return nc


_NC_CACHE = {}


def _build_cfg():
    K = int(os.environ.get("HB4_K", "8"))
    return (K,)


def _get_nc():
    key = _build_cfg()
    if key not in _NC_CACHE:
        (K,) = key
        _NC_CACHE[key] = build_nc(K=K)
    return _NC_CACHE[key]


def kernel(x, phis, interval):
    x = np.ascontiguousarray(x, dtype=np.float32)
    phis = np.ascontiguousarray(phis, dtype=np.float32)
    interval = np.ascontiguousarray(interval, dtype=np.float32)
    assert x.shape == (B, M) and phis.shape == (M, L) and interval.shape == (M,)

    nc = _get_nc()
    in_maps = []
    for c in range(N_CORES):
        shard = x[c * B_SHARD : (c + 1) * B_SHARD]
        in_maps.append(
            {
                "xt": np.ascontiguousarray(shard.T),
                "phis": phis,
                "interval": interval,
            }
        )
    res = run_bass_kernel_spmd(nc, in_maps, core_ids=list(range(N_CORES)))
    return np.concatenate(
        [np.asarray(res.results[c]["out"]).astype(np.float32) for c in range(N_CORES)],
        axis=0,
    )


# revision 7
# speedup vs baseline: 1.0288x; 1.0288x over previous
"""Trainium2 Bass kernel for histogram_binning — single fused DVE pass.

Reference (per element):
    d = x[k,i] - phis[i,j]
    out[k, i*L+j] = 0.5*cos(d)+0.5  if  -interval[i] < d <= interval[i]  else 0

Design (8 cores, data-parallel over batch; per-core out shard [128, 65536]):
  - fp16 output, host-upcast to f32 (rel-err gate 2e-2; fp16 + poly error
    ~8e-4).  Halves HBM write traffic vs f32.
  - Partition p owns feature pair {2p, 2p+1} (t=0/1), so each DMA line for
    (p, k) covers t*256+j = 512 fp16 = 1KB contiguous DRAM (512B lines are
    descriptor-bound; 1KB halves descriptor count -> DMA ~48us/core).
  - For |d| < interval[i] < 1:  0.5*cos(d)+0.5 = (1 - d^2/8)^2 + e,
    |e| <= d^4/192 < 4.6e-3, so ONE 7-stage custom DVE op per (chunk, t)
    computes window + value + mask + fp16 cast at 1 elem/cycle:
        d = Src1 - Src0   (x - phi; Src0 = phi bcast over k,
        u = d*d            Src1 = x bcast over j; stride-0 APs)
        c = u <= C0        (C0 = iv^2 [P,1]; boundary-rounding flips ~1e-7)
        t = One - C1*u     (C1 = 1/8)
        out = (c*t)*t -> fp16
    No ACT, no post pass, no GPSIMD (GPSIMD contends for the DVE SBUF
    port and poisons throughput).  DVE ~69us/core is the structural floor:
    combining the (i,j)- and (k,i)-indexed streams is inherently a 2-port
    DVE op, which caps at 1 elem/cycle.
"""

import os

import numpy as np

import concourse.bacc as bacc
import concourse.mybir as mybir
from concourse import dve_ops
from concourse.bass_utils import run_bass_kernel_spmd
from concourse.dve_spec import C0, C1, One, Spec, Src0, Src1, _has_src1, lower
from concourse.dve_uop import DveOpSpec
from concourse.tile import TileContext

B, M, L = 1024, 256, 256
N_CORES = 8
B_SHARD = B // N_CORES  # 128
F32 = mybir.dt.float32
F16 = mybir.dt.float16
P = 128
T = 2

_OPS_CACHE = {}


def _register_op(name, spec):
    """Register a custom DVE op under `name`, computing its uops sha."""
    if name in _OPS_CACHE:
        return _OPS_CACHE[name]
    for existing in dve_ops.OPS:
        if existing.name == name:
            _OPS_CACHE[name] = existing
            return existing
    if name not in dve_ops._SUB_OPCODE_FOR_NAME:
        row = max(dve_ops._SUB_OPCODE_FOR_NAME.values()) + 1
        assert row < 0x20, "no free custom-DVE opcode rows"
        dve_ops._SUB_OPCODE_FOR_NAME[name] = row
    shas = {}
    for ver in ("v3", "v4"):
        uops = lower(spec, ver=ver)
        shas[ver] = DveOpSpec(
            name=name,
            opcode=dve_ops.get_dve_sub_opcode(name),
            uops=uops,
            rd1_en=_has_src1(spec),
        ).sha(ver)
    op = dve_ops.DveOp(name, spec, subdim=False, uops_sha=shas)
    dve_ops.OPS.append(op)
    dve_ops.CUSTOM_DVE_SPECS[name] = spec
    _OPS_CACHE[name] = op
    return op


def _get_pwinpoly_op():
    """Window via squared compare (C0 = iv^2): d^2 <= iv^2 differs from the
    exact (-iv, iv] window only at fp32-rounding boundary flips (~1e-7 of
    elements) and at d == -iv exactly (measure zero)."""
    d = Src1 - Src0
    u = d * d
    cond = u <= C0
    t = One - C1 * u
    body = (cond * t) * t

    def _ref(in0, in1, s0, s1, imm2):
        f = np.float32
        dd = (in1 - in0).astype(np.float32)
        u = (dd * dd).astype(np.float32)
        m = u <= s0
        t = (f(1.0) - (f(s1) * u).astype(np.float32)).astype(np.float32)
        return ((m.astype(np.float32) * t) * t).astype(np.float32)

    return _register_op("PWINPOLY_ANT", Spec(body=body, reference=_ref))


def build_nc(
    K=8,
    num_devices=N_CORES,
    bufs=None,
    reps=1,
    timing=False,
):
    assert B_SHARD % K == 0
    n_chunks = B_SHARD // K

    nc = bacc.Bacc(
        "TRN2",
        target_bir_lowering=False,
        debug=False,
        enable_asserts=True,
        num_devices=num_devices,
    )
    xt_d = nc.dram_tensor("xt", [M, B_SHARD], F32, kind="ExternalInput")
    ph_d = nc.dram_tensor("phis", [M, L], F32, kind="ExternalInput")
    iv_d = nc.dram_tensor("interval", [M], F32, kind="ExternalInput")
    out_kind = "Internal" if timing else "ExternalOutput"
    y_d = nc.dram_tensor("out", [B_SHARD, M * L], F16, kind=out_kind)
    sink_d = (
        nc.dram_tensor("sink", [P, 1], F32, kind="ExternalOutput")
        if timing
        else None
    )
    # out[b, 512p + 256t + j] viewed as [p, b, (t j)]
    yr = y_d.ap().rearrange("b (p t j) -> p b (t j)", p=P, t=T, j=L)
    phr = ph_d.ap().rearrange("(p t) j -> p (t j)", t=T)
    ivr = iv_d.ap().rearrange("(p t) -> p t", t=T)
    xtr = xt_d.ap().rearrange("(p t) (ci k) -> p ci t k", t=T, k=K)

    pwin = _get_pwinpoly_op()

    if bufs is None:
        bufs = 4 if K <= 16 else 3
    with TileContext(nc) as tc:
        with (
            tc.tile_pool(name="const", bufs=1) as cpool,
            tc.tile_pool(name="owork", bufs=bufs) as opool,
        ):
            iv_t = cpool.tile([P, T], F32, tag="iv")
            nc.sync.dma_start(out=iv_t[:], in_=ivr)
            iv2_t = cpool.tile([P, T], F32, tag="iv2")
            nc.vector.tensor_tensor(
                out=iv2_t[:], in0=iv_t[:], in1=iv_t[:],
                op=mybir.AluOpType.mult,
            )
            ph_t = cpool.tile([P, T * L], F32, tag="ph")
            # scalar (ACT) HWDGE ring -> overlaps with iv/xk on the SP ring
            nc.scalar.dma_start(out=ph_t[:], in_=phr)
            xk_t = cpool.tile([P, T * B_SHARD], F32, tag="xk")
            xk_v = xk_t[:].rearrange("p (ci t k) -> p ci t k", t=T, k=K)
            for t in range(T):
                nc.sync.dma_start(out=xk_v[:, :, t, :], in_=xtr[:, :, t, :])

            def emit_chunk(k0, Kc):
                o = opool.tile([P, Kc * T * L], F16, tag="o")
                o4 = o[:].rearrange("p (k t j) -> p k t j", t=T, j=L)
                for t in range(T):
                    in0 = (
                        ph_t[:, t * L : (t + 1) * L]
                        .rearrange("p (o j) -> p o j", o=1)
                        .broadcast_to((P, Kc, L))
                    )
                    # xk_t layout is (ci, t, k) with the build-time K
                    ci0, r0 = divmod(k0, K)
                    assert r0 + Kc <= K, (k0, Kc)
                    sl = slice(
                        ci0 * T * K + t * K + r0,
                        ci0 * T * K + t * K + r0 + Kc,
                    )
                    in1 = (
                        xk_t[:, sl]
                        .rearrange("p (k o) -> p k o", o=1)
                        .broadcast_to((P, Kc, L))
                    )
                    nc.vector._custom_dve(
                        pwin,
                        out=o4[:, :, t, :],
                        in0=in0,
                        in1=in1,
                        s0=iv2_t[:, t : t + 1],
                        s1=0.125,
                    )
                nc.sync.dma_start(
                    out=yr[:, k0 : k0 + Kc, :], in_=o[:]
                )

            import contextlib

            loop_ctx = (
                tc.For_i(0, reps, 1, hint_engines=tuple(mybir.ALL_ENGINES))
                if reps > 1
                else contextlib.nullcontext()
            )
            # split the final chunk to shrink the trailing DMA
            chunk_list = [(ci * K, K) for ci in range(n_chunks - 1)]
            h2 = K // 2
            chunk_list += [((n_chunks - 1) * K, h2), ((n_chunks - 1) * K + h2, h2)]
            with loop_ctx:
                for k0, Kc in chunk_list:
                    emit_chunk(k0, Kc)
            if timing:
                nc.sync.dma_start(out=sink_d.ap(), in_=iv_t[:, 0:1])
    nc.compile()
    return nc


_NC_CACHE = {}


def _build_cfg():
    K = int(os.environ.get("HB4_K", "8"))
    return (K,)


def _get_nc():
    key = _build_cfg()
    if key not in _NC_CACHE:
        (K,) = key
        _NC_CACHE[key] = build_nc(K=K)
    return _NC_CACHE[key]


def kernel(x, phis, interval):
    x = np.ascontiguousarray(x, dtype=np.float32)
    phis = np.ascontiguousarray(phis, dtype=np.float32)
    interval = np.ascontiguousarray(interval, dtype=np.float32)
    assert x.shape == (B, M) and phis.shape == (M, L) and interval.shape == (M,)

    nc = _get_nc()
    in_maps = []
    for c in range(N_CORES):
        shard = x[c * B_SHARD : (c + 1) * B_SHARD]
        in_maps.append(
            {
                "xt": np.ascontiguousarray(shard.T),
                "phis": phis,
                "interval": interval,
            }
        )
    res = run_bass_kernel_spmd(nc, in_maps, core_ids=list(range(N_CORES)))
    return np.concatenate(
        [np.asarray(res.results[c]["out"]).astype(np.float32) for c in range(N_CORES)],
        axis=0,
    )
